# revision 24
# baseline (speedup 1.0000x reference)
"""Trainium2 Bass kernel for nn_InterfaceGraph (retrieval_knn).

Value-only formulation, K=5: the outputs depend only on each atom's
MINIMUM same-graph distance d2_min = |a|^2 - max_j (2 a.b_j - |b_j|^2).
The |a|^2 term is constant per row, so the device only computes
key_ij = 2 a.b_j - |b_j|^2 with a K=5 fp16 matmul per 128-row tile
(rows: 2a_x, 2a_y, 2a_z paired with b_x, b_y, b_z; two -1 rows paired
with the hi/lo fp16 split of |b|^2).  Positions are fp16-rounded; the
resulting distance error is <= ~0.12 absolute, absorbed by the host
recompute bands.  vs the K=21 exact-d2 formulation this cuts input DMA
bytes ~4x and PE contraction rows ~4x; DVE tensor_reduce (1x-only, the
only paged-reduce path on TRN2) remains the compute wall at ~1.04
ns/column + 125 ns/instruction PSUM-access bubble.

Both directions (a->b and b->a) share one tile stream: row-tiles sorted
by padded column width, dealt round-robin to the 8 cores, grouped into
4-slot PSUM slabs (one bank each, 2-buffer rotation).  Inputs ride the
two hardware DGE queues as one small chunk per slab, in slab order;
slab 0 is split across BOTH queues' first DMAs so its four matmuls gate
only on first-in-queue transfers (the first matmul's semaphore wait is
tick-coalesced per queue).  Measured ~27.5-28.1 us HW exec (baseline
31.9): ~6.7 us fixed NEFF preamble, ~2.2 us first-data chain, ~12.0 us
DVE-paced reduce (back-to-back MAX slabs, <0.2 us total gaps), ~1.8 us
transpose+copy+output-DMA chain, ~3.5 us runtime teardown.  Rejected by
measurement: 8-slot 2-per-bank slabs (PE matmul burst at mid p-state
overflows the previous MAX window, stalling the DVE), split output DMA,
split end-transpose (transpose outputs must land at PSUM partition 0).

Host epilogue: d = sqrt(|a|^2 - maxkey); rows with d < 7.0 or
|d-10| < 0.45 are recomputed exactly (vectorized fp64 argmin + fp32
norm per graph, matching the reference formula), so small-d relative
error and the d<10 interface-cutoff comparisons are exact; residue
segment-max mask + mutation OR on host as before.
"""

import numpy as np


NCORES = 8
G = 64
NUM_RESIDUES = 2048
CUTOFF = np.float32(10.0)
BIG = np.float32(60000.0)   # pad-column key = -BIG; valid keys stay > -4000
K = 5
BANKS = 4          # psum banks per slab tile

RECOMP_D = 7.0     # host recompute band: d < RECOMP_D
BAND10 = 0.45      # and |d - 10| < BAND10

PROFILE = False
LAST_EXEC_NS = None

F16 = np.float16

_prog_cache = {}


def _round_up(x, m):
    return (x + m - 1) // m * m


def _install_ntff_hook():
    import sys
    import types
    if 'antenv.axon_hooks' in sys.modules:
        return
    from trn_agent_boot.trn_boot import _ntff_profile_via_ctypes
    hook = _ntff_profile_via_ctypes('/opt/axon/libaxon_pjrt.so')
    mod = types.ModuleType('antenv.axon_hooks')
    mod.get_axon_ntff_profile_hook = lambda: hook
    sys.modules['antenv.axon_hooks'] = mod


class _Slab:
    __slots__ = ("start", "count", "W", "nsub", "lhs_loc", "rhs_loc")

    def __init__(self, start, count, W, nsub):
        self.start = start      # first slot (= val column) index
        self.count = count      # 4 * nsub slots
        self.W = W
        self.nsub = nsub        # slots per psum bank (1 or 2)
        self.lhs_loc = None     # (chunk, offset) of count*128 lhs cols
        self.rhs_loc = None     # per local slot: (chunk, offset)


class _Geom:
    """Unified tile list over BOTH directions.

    A tile is 128 consecutive row-atoms of one graph scanning that
    graph's full opposite-side column block.  side 0: rows=a cols=b;
    side 1: rows=b cols=a.  Sorted by padded column width desc, slot s
    holds tiles [8s:8s+8] across the 8 cores; missing entries are dummy
    tiles (zero lhs).
    """

    def __init__(self, na, nb):
        tiles = []          # (side, graph, row_chunk, W)
        for g in range(G):
            Wb = max(8, _round_up(int(nb[g]), 4))
            for r in range(-(-int(na[g]) // 128)):
                tiles.append((0, g, r, Wb))
            Wa = max(8, _round_up(int(na[g]), 4))
            for r in range(-(-int(nb[g]) // 128)):
                tiles.append((1, g, r, Wa))
        tiles.sort(key=lambda t: (-t[3], t[0], t[1], t[2]))
        self.nslots = _round_up(-(-len(tiles) // NCORES), BANKS)
        tiles += [(-1, -1, 0, 8)] * (self.nslots * NCORES - len(tiles))
        # slot s, core c -> tiles[s*8 + c]
        self.tile = [[tiles[s * NCORES + c] for c in range(NCORES)]
                     for s in range(self.nslots)]
        slotW = [int(_round_up(max(self.tile[s][c][3]
                                   for c in range(NCORES)), 4))
                 for s in range(self.nslots)]
        # 4-slot slabs (one psum bank each).  8-slot 2-per-bank packing
        # measured worse: the 8-matmul burst exceeds the previous MAX
        # window at PE mid p-state, stalling the DVE.
        self.slabs = []
        for i in range(0, self.nslots, 4):
            w4 = max(slotW[i:i + 4])
            assert w4 <= 512
            self.slabs.append(_Slab(i, 4, w4, 1))
        # chunks: first slab's first slot leads; then its other slots;
        # then slabs batched ~5k columns per chunk.
        self._build_chunks()

    def _build_chunks(self):
        self.chunk_size = []

        def new_chunk():
            self.chunk_size.append(0)

        def put(n):
            ci = len(self.chunk_size) - 1
            off = self.chunk_size[ci]
            self.chunk_size[ci] += n
            return (ci, off)

        # slab 0 split across two small chunks: the first compute's
        # (tick-coalesced) wait covers the first two sync-queue DMAs, so
        # both must be tiny for the earliest possible matmul start.
        s0 = self.slabs[0]
        new_chunk()
        s0.lhs_loc = put(s0.count * 128)
        s0.rhs_loc = [put(s0.W)]
        new_chunk()
        for _ in range(1, s0.count):
            s0.rhs_loc.append(put(s0.W))
        for sl in self.slabs[1:]:
            new_chunk()
            sl.lhs_loc = put(sl.count * 128)
            sl.rhs_loc = [put(sl.W) for _ in range(sl.count)]
        self.chunk_base = np.concatenate(
            [[0], np.cumsum(self.chunk_size)]).astype(int)
        self.T = int(self.chunk_base[-1])

    def key(self):
        return (self.nslots,
                tuple((sl.start, sl.count, sl.W) for sl in self.slabs))


def _build_program(geom):
    from contextlib import ExitStack

    import concourse.bacc as bacc
    import concourse.mybir as mybir
    import concourse.tile as tile
    from concourse import masks

    f32 = mybir.dt.float32
    f16 = mybir.dt.float16

    nc = bacc.Bacc("TRN2", target_bir_lowering=False, debug=False,
                   enable_asserts=True, num_devices=NCORES)

    inp = nc.dram_tensor("inp", [K, geom.T], f16, kind="ExternalInput").ap()
    valT = nc.dram_tensor("valT", [geom.nslots, 128], f32,
                          kind="ExternalOutput").ap()

    nchunks = len(geom.chunk_size)

    with tile.TileContext(nc) as tc:
        with ExitStack() as ctx:
            const = ctx.enter_context(tc.tile_pool(name="const", bufs=1))
            psum = ctx.enter_context(
                tc.tile_pool(name="psum", bufs=2, space="PSUM"))

            ch_sb = [const.tile([K, geom.chunk_size[ci]], f16,
                                tag=f"ch{ci}", name=f"ch{ci}")
                     for ci in range(nchunks)]
            val_sb = const.tile([128, geom.nslots], f32, tag="val")
            valT_sb = const.tile([geom.nslots, 128], f32, tag="valT")
            ident = const.tile([128, 128], f32, tag="ident")

            # chunk-ordered input DMAs; slab 0's two small chunks lead
            # BOTH queues (s0a on sync, s0b on scalar) so its four
            # matmuls gate only on first-in-queue transfers; later slabs
            # alternate queues in slab order.
            for ci in range(nchunks):
                q = nc.sync if ci % 2 == 0 else nc.scalar
                b0 = int(geom.chunk_base[ci])
                q.dma_start(ch_sb[ci][:],
                            inp[:, b0:b0 + geom.chunk_size[ci]])

            # identity for the end transpose: built on gpsimd, off the
            # critical path (needed only after the last reduce)
            masks.make_identity(nc, ident[:])

            # preload the activation table during the DMA window so the
            # end-chain ScalarE copy doesn't stall on it
            nc.scalar.activation(ident[0:128, 0:8], ident[0:128, 0:8],
                                 mybir.ActivationFunctionType.Copy)

            def emit_matmuls(sl):
                W = sl.nsub * sl.W
                ps = psum.tile([128, BANKS, 512], f32, tag="ps", name="ps")
                lci, loff = sl.lhs_loc
                for j in range(sl.count):
                    bank, sub = divmod(j, sl.nsub)
                    rci, roff = sl.rhs_loc[j]
                    nc.tensor.matmul(
                        ps[:, bank, sub * sl.W:(sub + 1) * sl.W],
                        ch_sb[lci][:, loff + j * 128:loff + (j + 1) * 128],
                        ch_sb[rci][:, roff:roff + sl.W],
                        start=True, stop=True)
                return ps

            def emit_reduce(sl, ps):
                src = ps[:, :, 0:sl.nsub * sl.W]
                if sl.nsub > 1:
                    src = src.rearrange("p b (s w) -> p b s w", s=sl.nsub)
                nc.vector.reduce_max(
                    val_sb[:, sl.start:sl.start + sl.count],
                    src, axis=mybir.AxisListType.X)

            for sl in geom.slabs:
                ps = emit_matmuls(sl)
                emit_reduce(sl, ps)

            # transpose [128, nslots] -> [nslots, 128] so the output DMA
            # uses nslots fat descriptors instead of 128 tiny ones.  The
            # output is split at a 32-slot boundary: slots [0:cut] only
            # need the first 8 slabs, so their transpose (PE), copy
            # (ScalarE -- keeps the DVE free) and DMA descriptor
            # generation all hide under the last reduce; the tail chain
            # is a small [128, tail] transpose + DVE copy + short DMA.
            ns = geom.nslots
            cut = (ns // 32) * 32
            if cut in (0, ns):
                cut = ns - 4
            pst = psum.tile([128, BANKS, 512], f32, tag="ps", name="pst")
            nc.tensor.transpose(
                pst[0:cut, 0, 0:128], val_sb[:, 0:cut], ident[:])
            nc.scalar.activation(valT_sb[0:cut, :], pst[0:cut, 0, 0:128],
                                 mybir.ActivationFunctionType.Copy)
            nc.sync.dma_start(valT[0:cut, :], valT_sb[0:cut, :])
            nc.tensor.transpose(
                pst[0:ns - cut, 1, 0:128], val_sb[:, cut:ns], ident[:])
            nc.vector.tensor_copy(valT_sb[cut:ns, :],
                                  pst[0:ns - cut, 1, 0:128])
            nc.sync.dma_start(valT[cut:ns, :], valT_sb[cut:ns, :])

    nc.compile()
    return nc


def _pack(geom, pa16, pb16, b2a, b2b, sa, sb, core):
    """Packed input [K, T] fp16 for one core.

    pa16/pb16: fp16-rounded positions as float32 [N, 3].
    b2a/b2b:   per-atom |pos16|^2 in float32.
    """
    inp = np.zeros((K, geom.T), np.float32)
    pos = (pa16, pb16)
    b2 = (b2a, b2b)
    starts = (sa, sb)
    cbase = geom.chunk_base
    for sl in geom.slabs:
        lci, loff = sl.lhs_loc
        lhs0 = int(cbase[lci]) + loff
        for j in range(sl.count):
            s = sl.start + j
            rci, roff = sl.rhs_loc[j]
            off = int(cbase[rci]) + roff
            inp[3, off:off + sl.W] = BIG   # pad cols give key = -BIG
            side, g, r, _ = geom.tile[s][core]
            if g < 0:
                continue
            s_row = starts[side]
            s_col = starts[1 - side]
            p = pos[side][s_row[g] + 128 * r:
                          min(s_row[g] + 128 * (r + 1), s_row[g + 1])]
            n = p.shape[0]
            q = pos[1 - side][s_col[g]:s_col[g + 1]]
            q2 = b2[1 - side][s_col[g]:s_col[g + 1]]
            m = q.shape[0]
            inp[0:3, off:off + m] = q.T
            v1 = q2.astype(F16).astype(np.float32)
            v2 = (q2 - v1).astype(F16).astype(np.float32)
            inp[3, off:off + m] = v1
            inp[4, off:off + m] = v2
            lb = lhs0 + j * 128
            inp[0:3, lb:lb + n] = 2.0 * p.T
            inp[3, lb:lb + n] = -1.0
            inp[4, lb:lb + n] = -1.0
    return inp.astype(F16)


def kernel(pos_a, pos_b, node2graph_a, node2graph_b,
           atom2residue_a, atom2residue_b, is_mutation):
    global LAST_EXEC_NS

    from concourse.bass_utils import run_bass_kernel_spmd

    pos_a = np.asarray(pos_a, dtype=np.float32)
    pos_b = np.asarray(pos_b, dtype=np.float32)
    node2graph_a = np.asarray(node2graph_a)
    node2graph_b = np.asarray(node2graph_b)
    atom2residue_a = np.asarray(atom2residue_a)
    atom2residue_b = np.asarray(atom2residue_b)
    is_mutation = np.asarray(is_mutation)

    sa = np.searchsorted(node2graph_a, np.arange(G + 1)).astype(np.int64)
    sb = np.searchsorted(node2graph_b, np.arange(G + 1)).astype(np.int64)
    na = np.diff(sa)
    nb = np.diff(sb)
    assert na.min() > 0 and nb.min() > 0, "empty graph block not supported"

    geom = _Geom(na, nb)
    key = geom.key()
    if key not in _prog_cache:
        _prog_cache[key] = _build_program(geom)
    nc = _prog_cache[key]

    pa16 = pos_a.astype(F16).astype(np.float32)
    pb16 = pos_b.astype(F16).astype(np.float32)
    b2a = (pa16 * pa16).sum(-1).astype(np.float32)
    b2b = (pb16 * pb16).sum(-1).astype(np.float32)

    in_maps = []
    for c in range(NCORES):
        in_maps.append({"inp": _pack(geom, pa16, pb16, b2a, b2b,
                                     sa, sb, c)})

    if PROFILE:
        _install_ntff_hook()
    res = run_bass_kernel_spmd(nc, in_maps, list(range(NCORES)),
                               trace=bool(PROFILE))
    if PROFILE:
        LAST_EXEC_NS = res.exec_time_ns

    key_a = np.empty(pos_a.shape[0], np.float64)
    key_b = np.empty(pos_b.shape[0], np.float64)
    starts = (sa, sb)
    keys = (key_a, key_b)
    for c in range(NCORES):
        v = res.results[c]["valT"]
        for s in range(geom.nslots):
            side, g, r, _ = geom.tile[s][c]
            if g < 0:
                continue
            s_row = starts[side]
            lo = s_row[g] + 128 * r
            hi = min(s_row[g] + 128 * (r + 1), s_row[g + 1])
            keys[side][lo:hi] = v[s, 0:hi - lo]

    # d2_min = |a16|^2 - max_j key
    norm2a = (pa16.astype(np.float64) ** 2).sum(-1)
    norm2b = (pb16.astype(np.float64) ** 2).sum(-1)
    d2_a = norm2a - key_a
    d2_b = norm2b - key_b

    def epilogue(d2dev, pos_row, pos_col, s_col, n2row):
        dist = np.sqrt(np.maximum(d2dev, 0.0)).astype(np.float32)
        flagged = (dist < RECOMP_D) | (np.abs(dist - np.float32(10.0))
                                       < BAND10)
        idx_all = np.where(flagged)[0]
        if idx_all.size:
            gids = n2row[idx_all]
            for g in np.unique(gids):
                idx = idx_all[gids == g]
                Q = pos_col[s_col[g]:s_col[g + 1]]
                P = pos_row[idx].astype(np.float64)
                d2 = ((P[:, None, :] - Q[None, :, :].astype(np.float64))
                      ** 2).sum(-1)
                j = np.argmin(d2, axis=1)
                diff = pos_row[idx] - Q[j]
                dist[idx] = np.sqrt((diff * diff).sum(-1,
                                                      dtype=np.float32))
        return dist

    dist_a = epilogue(d2_a, pos_a, pos_b, sb, node2graph_a)
    dist_b = epilogue(d2_b, pos_b, pos_a, sa, node2graph_b)

    def iface_mask(dist, atom2residue):
        is_if = (dist < CUTOFF).astype(np.int32)
        res_max = np.zeros(NUM_RESIDUES, dtype=np.int32)
        np.maximum.at(res_max, atom2residue, is_if)
        return res_max[atom2residue] > 0

    mask_a = iface_mask(dist_a, atom2residue_a)
    mask_b = iface_mask(dist_b, atom2residue_b)
    mask = np.concatenate([mask_a, mask_b]) | is_mutation.astype(bool)
    dists = np.concatenate([dist_a, dist_b]).astype(np.float32)
    return mask, dists


# revision 26
# speedup vs baseline: 1.1610x; 1.1610x over previous
"""Trainium2 Bass kernel for nn_InterfaceGraph (retrieval_knn).

Value-only formulation, K=5: the outputs depend only on each atom's
MINIMUM same-graph distance d2_min = |a|^2 - max_j (2 a.b_j - |b_j|^2).
The |a|^2 term is constant per row, so the device only computes
key_ij = 2 a.b_j - |b_j|^2 with a K=5 fp16 matmul per 128-row tile
(rows: 2a_x, 2a_y, 2a_z paired with b_x, b_y, b_z; two -1 rows paired
with the hi/lo fp16 split of |b|^2).  Positions are fp16-rounded; the
resulting distance error is <= ~0.12 absolute, absorbed by the host
recompute bands.  vs the K=21 exact-d2 formulation this cuts input DMA
bytes ~4x and PE contraction rows ~4x; DVE tensor_reduce (1x-only, the
only paged-reduce path on TRN2) remains the compute wall at ~1.04
ns/column + 125 ns/instruction PSUM-access bubble.

Both directions (a->b and b->a) share one tile stream: row-tiles sorted
by padded column width, dealt round-robin to the 8 cores, grouped into
4-slot PSUM slabs (one bank each, 2-buffer rotation).  Inputs ride the
two hardware DGE queues as one small chunk per slab, in slab order;
slab 0 is split across BOTH queues' first DMAs so its four matmuls gate
only on first-in-queue transfers (the first matmul's semaphore wait is
tick-coalesced per queue).  Measured ~27.5-28.1 us HW exec (baseline
31.9): ~6.7 us fixed NEFF preamble, ~2.2 us first-data chain, ~12.0 us
DVE-paced reduce (back-to-back MAX slabs, <0.2 us total gaps), ~1.8 us
transpose+copy+output-DMA chain, ~3.5 us runtime teardown.  Rejected by
measurement: 8-slot 2-per-bank slabs (PE matmul burst at mid p-state
overflows the previous MAX window, stalling the DVE), split output DMA,
split end-transpose (transpose outputs must land at PSUM partition 0).

Host epilogue: d = sqrt(|a|^2 - maxkey); rows with d < 7.0 or
|d-10| < 0.45 are recomputed exactly (vectorized fp64 argmin + fp32
norm per graph, matching the reference formula), so small-d relative
error and the d<10 interface-cutoff comparisons are exact; residue
segment-max mask + mutation OR on host as before.
"""

import numpy as np


NCORES = 8
G = 64
NUM_RESIDUES = 2048
CUTOFF = np.float32(10.0)
BIG = np.float32(60000.0)   # pad-column key = -BIG; valid keys stay > -4000
K = 5
BANKS = 4          # psum banks per slab tile

RECOMP_D = 7.0     # host recompute band: d < RECOMP_D
BAND10 = 0.45      # and |d - 10| < BAND10

PROFILE = False
LAST_EXEC_NS = None

F16 = np.float16

_prog_cache = {}


def _round_up(x, m):
    return (x + m - 1) // m * m


def _install_ntff_hook():
    import sys
    import types
    if 'antenv.axon_hooks' in sys.modules:
        return
    from trn_agent_boot.trn_boot import _ntff_profile_via_ctypes
    hook = _ntff_profile_via_ctypes('/opt/axon/libaxon_pjrt.so')
    mod = types.ModuleType('antenv.axon_hooks')
    mod.get_axon_ntff_profile_hook = lambda: hook
    sys.modules['antenv.axon_hooks'] = mod


class _Slab:
    __slots__ = ("start", "count", "W", "nsub", "lhs_loc", "rhs_loc")

    def __init__(self, start, count, W, nsub):
        self.start = start      # first slot (= val column) index
        self.count = count      # 4 * nsub slots
        self.W = W
        self.nsub = nsub        # slots per psum bank (1 or 2)
        self.lhs_loc = None     # (chunk, offset) of count*128 lhs cols
        self.rhs_loc = None     # per local slot: (chunk, offset)


class _Geom:
    """Unified tile list over BOTH directions.

    A tile is 128 consecutive row-atoms of one graph scanning that
    graph's full opposite-side column block.  side 0: rows=a cols=b;
    side 1: rows=b cols=a.  Sorted by padded column width desc, slot s
    holds tiles [8s:8s+8] across the 8 cores; missing entries are dummy
    tiles (zero lhs).
    """

    def __init__(self, na, nb):
        tiles = []          # (side, graph, row_chunk, W)
        for g in range(G):
            Wb = max(8, _round_up(int(nb[g]), 4))
            for r in range(-(-int(na[g]) // 128)):
                tiles.append((0, g, r, Wb))
            Wa = max(8, _round_up(int(na[g]), 4))
            for r in range(-(-int(nb[g]) // 128)):
                tiles.append((1, g, r, Wa))
        tiles.sort(key=lambda t: (-t[3], t[0], t[1], t[2]))
        self.nslots = _round_up(-(-len(tiles) // NCORES), BANKS)
        tiles += [(-1, -1, 0, 8)] * (self.nslots * NCORES - len(tiles))
        # slot s, core c -> tiles[s*8 + c]
        self.tile = [[tiles[s * NCORES + c] for c in range(NCORES)]
                     for s in range(self.nslots)]
        slotW = [int(_round_up(max(self.tile[s][c][3]
                                   for c in range(NCORES)), 4))
                 for s in range(self.nslots)]
        # 4-slot slabs (one psum bank each).  8-slot 2-per-bank packing
        # measured worse: the 8-matmul burst exceeds the previous MAX
        # window at PE mid p-state, stalling the DVE.
        self.slabs = []
        for i in range(0, self.nslots, 4):
            w4 = max(slotW[i:i + 4])
            assert w4 <= 512
            self.slabs.append(_Slab(i, 4, w4, 1))
        # chunks: first slab's first slot leads; then its other slots;
        # then slabs batched ~5k columns per chunk.
        self._build_chunks()

    def _build_chunks(self):
        self.chunk_size = []

        def new_chunk():
            self.chunk_size.append(0)

        def put(n):
            ci = len(self.chunk_size) - 1
            off = self.chunk_size[ci]
            self.chunk_size[ci] += n
            return (ci, off)

        # slab 0 split across two small chunks: the first compute's
        # (tick-coalesced) wait covers the first two sync-queue DMAs, so
        # both must be tiny for the earliest possible matmul start.
        s0 = self.slabs[0]
        new_chunk()
        s0.lhs_loc = put(s0.count * 128)
        s0.rhs_loc = [put(s0.W)]
        new_chunk()
        for _ in range(1, s0.count):
            s0.rhs_loc.append(put(s0.W))
        for sl in self.slabs[1:]:
            new_chunk()
            sl.lhs_loc = put(sl.count * 128)
            sl.rhs_loc = [put(sl.W) for _ in range(sl.count)]
        self.chunk_base = np.concatenate(
            [[0], np.cumsum(self.chunk_size)]).astype(int)
        self.T = int(self.chunk_base[-1])

    def key(self):
        return (self.nslots,
                tuple((sl.start, sl.count, sl.W) for sl in self.slabs))


def _build_program(geom):
    from contextlib import ExitStack

    import concourse.bacc as bacc
    import concourse.mybir as mybir
    import concourse.tile as tile
    from concourse import masks

    f32 = mybir.dt.float32
    f16 = mybir.dt.float16

    nc = bacc.Bacc("TRN2", target_bir_lowering=False, debug=False,
                   enable_asserts=True, num_devices=NCORES)

    inp = nc.dram_tensor("inp", [K, geom.T], f16, kind="ExternalInput").ap()
    valT = nc.dram_tensor("valT", [geom.nslots, 128], f32,
                          kind="ExternalOutput").ap()

    nchunks = len(geom.chunk_size)

    with tile.TileContext(nc) as tc:
        with ExitStack() as ctx:
            const = ctx.enter_context(tc.tile_pool(name="const", bufs=1))
            psum = ctx.enter_context(
                tc.tile_pool(name="psum", bufs=2, space="PSUM"))

            ch_sb = [const.tile([K, geom.chunk_size[ci]], f16,
                                tag=f"ch{ci}", name=f"ch{ci}")
                     for ci in range(nchunks)]
            val_sb = const.tile([128, geom.nslots], f32, tag="val")
            valT_sb = const.tile([geom.nslots, 128], f32, tag="valT")
            ident = const.tile([128, 128], f32, tag="ident")

            # chunk-ordered input DMAs; slab 0's two small chunks lead
            # BOTH queues (s0a on sync, s0b on scalar) so its four
            # matmuls gate only on first-in-queue transfers; later slabs
            # alternate queues in slab order.
            for ci in range(nchunks):
                q = nc.sync if ci % 2 == 0 else nc.scalar
                b0 = int(geom.chunk_base[ci])
                q.dma_start(ch_sb[ci][:],
                            inp[:, b0:b0 + geom.chunk_size[ci]])

            # identity for the end transpose: built on gpsimd, off the
            # critical path (needed only after the last reduce)
            masks.make_identity(nc, ident[:])

            def emit_matmuls(sl):
                W = sl.nsub * sl.W
                ps = psum.tile([128, BANKS, 512], f32, tag="ps", name="ps")
                lci, loff = sl.lhs_loc
                for j in range(sl.count):
                    bank, sub = divmod(j, sl.nsub)
                    rci, roff = sl.rhs_loc[j]
                    nc.tensor.matmul(
                        ps[:, bank, sub * sl.W:(sub + 1) * sl.W],
                        ch_sb[lci][:, loff + j * 128:loff + (j + 1) * 128],
                        ch_sb[rci][:, roff:roff + sl.W],
                        start=True, stop=True)
                return ps

            def emit_reduce(sl, ps):
                src = ps[:, :, 0:sl.nsub * sl.W]
                if sl.nsub > 1:
                    src = src.rearrange("p b (s w) -> p b s w", s=sl.nsub)
                nc.vector.reduce_max(
                    val_sb[:, sl.start:sl.start + sl.count],
                    src, axis=mybir.AxisListType.X)

            for sl in geom.slabs:
                ps = emit_matmuls(sl)
                emit_reduce(sl, ps)

            # transpose [128, nslots] -> [nslots, 128] so the output DMA
            # uses nslots fat descriptors instead of 128 tiny ones.
            # (A split output chain overlapping the last reduce measured
            # neutral: DMA descriptor generation is fixed ~0.8us
            # regardless of row count, and the remaining small transpose
            # is FD=128-bound, so the tail doesn't shrink.)
            pst = psum.tile([128, BANKS, 512], f32, tag="ps", name="pst")
            nc.tensor.transpose(
                pst[0:geom.nslots, 0, 0:128], val_sb[:], ident[:])
            nc.vector.tensor_copy(valT_sb[:], pst[0:geom.nslots, 0, 0:128])
            nc.sync.dma_start(valT[:], valT_sb[:])

    nc.compile()
    return nc


def _pack(geom, pa16, pb16, b2a, b2b, sa, sb, core):
    """Packed input [K, T] fp16 for one core.

    pa16/pb16: fp16-rounded positions as float32 [N, 3].
    b2a/b2b:   per-atom |pos16|^2 in float32.
    """
    inp = np.zeros((K, geom.T), np.float32)
    pos = (pa16, pb16)
    b2 = (b2a, b2b)
    starts = (sa, sb)
    cbase = geom.chunk_base
    for sl in geom.slabs:
        lci, loff = sl.lhs_loc
        lhs0 = int(cbase[lci]) + loff
        for j in range(sl.count):
            s = sl.start + j
            rci, roff = sl.rhs_loc[j]
            off = int(cbase[rci]) + roff
            inp[3, off:off + sl.W] = BIG   # pad cols give key = -BIG
            side, g, r, _ = geom.tile[s][core]
            if g < 0:
                continue
            s_row = starts[side]
            s_col = starts[1 - side]
            p = pos[side][s_row[g] + 128 * r:
                          min(s_row[g] + 128 * (r + 1), s_row[g + 1])]
            n = p.shape[0]
            q = pos[1 - side][s_col[g]:s_col[g + 1]]
            q2 = b2[1 - side][s_col[g]:s_col[g + 1]]
            m = q.shape[0]
            inp[0:3, off:off + m] = q.T
            v1 = q2.astype(F16).astype(np.float32)
            v2 = (q2 - v1).astype(F16).astype(np.float32)
            inp[3, off:off + m] = v1
            inp[4, off:off + m] = v2
            lb = lhs0 + j * 128
            inp[0:3, lb:lb + n] = 2.0 * p.T
            inp[3, lb:lb + n] = -1.0
            inp[4, lb:lb + n] = -1.0
    return inp.astype(F16)


def kernel(pos_a, pos_b, node2graph_a, node2graph_b,
           atom2residue_a, atom2residue_b, is_mutation):
    global LAST_EXEC_NS

    from concourse.bass_utils import run_bass_kernel_spmd

    pos_a = np.asarray(pos_a, dtype=np.float32)
    pos_b = np.asarray(pos_b, dtype=np.float32)
    node2graph_a = np.asarray(node2graph_a)
    node2graph_b = np.asarray(node2graph_b)
    atom2residue_a = np.asarray(atom2residue_a)
    atom2residue_b = np.asarray(atom2residue_b)
    is_mutation = np.asarray(is_mutation)

    sa = np.searchsorted(node2graph_a, np.arange(G + 1)).astype(np.int64)
    sb = np.searchsorted(node2graph_b, np.arange(G + 1)).astype(np.int64)
    na = np.diff(sa)
    nb = np.diff(sb)
    assert na.min() > 0 and nb.min() > 0, "empty graph block not supported"

    geom = _Geom(na, nb)
    key = geom.key()
    if key not in _prog_cache:
        _prog_cache[key] = _build_program(geom)
    nc = _prog_cache[key]

    pa16 = pos_a.astype(F16).astype(np.float32)
    pb16 = pos_b.astype(F16).astype(np.float32)
    b2a = (pa16 * pa16).sum(-1).astype(np.float32)
    b2b = (pb16 * pb16).sum(-1).astype(np.float32)

    in_maps = []
    for c in range(NCORES):
        in_maps.append({"inp": _pack(geom, pa16, pb16, b2a, b2b,
                                     sa, sb, c)})

    if PROFILE:
        _install_ntff_hook()
    res = run_bass_kernel_spmd(nc, in_maps, list(range(NCORES)),
                               trace=bool(PROFILE))
    if PROFILE:
        LAST_EXEC_NS = res.exec_time_ns

    key_a = np.empty(pos_a.shape[0], np.float64)
    key_b = np.empty(pos_b.shape[0], np.float64)
    starts = (sa, sb)
    keys = (key_a, key_b)
    for c in range(NCORES):
        v = res.results[c]["valT"]
        for s in range(geom.nslots):
            side, g, r, _ = geom.tile[s][c]
            if g < 0:
                continue
            s_row = starts[side]
            lo = s_row[g] + 128 * r
            hi = min(s_row[g] + 128 * (r + 1), s_row[g + 1])
            keys[side][lo:hi] = v[s, 0:hi - lo]

    # d2_min = |a16|^2 - max_j key
    norm2a = (pa16.astype(np.float64) ** 2).sum(-1)
    norm2b = (pb16.astype(np.float64) ** 2).sum(-1)
    d2_a = norm2a - key_a
    d2_b = norm2b - key_b

    def epilogue(d2dev, pos_row, pos_col, s_col, n2row):
        dist = np.sqrt(np.maximum(d2dev, 0.0)).astype(np.float32)
        flagged = (dist < RECOMP_D) | (np.abs(dist - np.float32(10.0))
                                       < BAND10)
        idx_all = np.where(flagged)[0]
        if idx_all.size:
            gids = n2row[idx_all]
            for g in np.unique(gids):
                idx = idx_all[gids == g]
                Q = pos_col[s_col[g]:s_col[g + 1]]
                P = pos_row[idx].astype(np.float64)
                d2 = ((P[:, None, :] - Q[None, :, :].astype(np.float64))
                      ** 2).sum(-1)
                j = np.argmin(d2, axis=1)
                diff = pos_row[idx] - Q[j]
                dist[idx] = np.sqrt((diff * diff).sum(-1,
                                                      dtype=np.float32))
        return dist

    dist_a = epilogue(d2_a, pos_a, pos_b, sb, node2graph_a)
    dist_b = epilogue(d2_b, pos_b, pos_a, sa, node2graph_b)

    def iface_mask(dist, atom2residue):
        is_if = (dist < CUTOFF).astype(np.int32)
        res_max = np.zeros(NUM_RESIDUES, dtype=np.int32)
        np.maximum.at(res_max, atom2residue, is_if)
        return res_max[atom2residue] > 0

    mask_a = iface_mask(dist_a, atom2residue_a)
    mask_b = iface_mask(dist_b, atom2residue_b)
    mask = np.concatenate([mask_a, mask_b]) | is_mutation.astype(bool)
    dists = np.concatenate([dist_a, dist_b]).astype(np.float32)
    return mask, dists
